# revision 34
# baseline (speedup 1.0000x reference)
"""Bass/Trainium2 kernel for CausalSelfAttention (B=8, T=1024, C=768, H=12).

Sharding: data-parallel over batch. 8 cores, one batch element per core.
No collectives. Each core runs an identical SPMD program on its own slice.

Per-core layouts (host-prepared):
  xT   [768, 1024] bf16   x[b].T
  wqk  [768, 1536] bf16   W_attn[:, :1536], Q columns pre-scaled by 1/sqrt(64)
  wv   [768, 768]  bf16   W_attn[:, 1536:]
  wp   [768, 768]  bf16   W_proj
  bqk  [128, 12]  f32     b_attn[:1536] per-tile columns (Q part pre-scaled)
  bv   [128, 768] f32     b_attn[1536:] broadcast over partitions
  bp   [128, 768] f32     b_proj broadcast over partitions
  qm   [128, 8]   f32     query_mask as per-partition columns per q-tile
  dmL  [128, 8, 128] bf16 diagonal-block ADDITIVE masks (0 / -60000), with
                          partition = query-within-block, free = key-within
  id128 [128, 128] bf16   identity (rhs of the mask-add matmuls)
Output: y [1024, 768] f32 per core.

Schedule (keep the tensor engine continuously busy so its DVFS p-state
ramps to full clock and never drops):
  - fine-grained priority DMA: first-matmul deps (wqk m-block 0, xT j0)
    land first; the PE starts ~2us in
  - qk projections for (m=0,6), V for all 8 key tiles, then pairs
    pr=0..5: [emit_qk(pr+1) | flush norm(pr-1,1)], attention(pr,0),
    [emit_qk(7+pr) | flush norm(pr,0)], attention(pr,1)
  - attention software-pipelines PV one key-tile behind the exp so the
    PE never waits on the scalar engine
  - causal masking of diagonal blocks is ADDITIVE, folded into the S
    psum by one extra 128-row matmul (dmL^T @ I) per head -- no
    cross-engine op sits between exp and PV
  - both heads of a pair share one 2-bank [128,1024] S psum; one strided
    exp covers both heads (halves scalar-engine instruction count)
  - softmax normalization (copy sums -> PE broadcast -> reciprocal ->
    multiply) is DEFERRED and flushed at the next qk-projection window,
    when the tensor queue holds >1us of work, so the broadcast matmul
    and the psO buffer rotation never stall the PE
  - qkv bias-adds on DVE to keep the scalar engine exp-only
  - output projection qt 0-3 interleaved into the last pair's sbi=1
    attention; only qt 4-7 in the tail
"""

import sys

if "/opt/trn_rl_repo" not in sys.path:
    sys.path.insert(0, "/opt/trn_rl_repo")

import numpy as np
import ml_dtypes

import concourse.bass as bass
import concourse.bacc as bacc
import concourse.mybir as mybir
import concourse.tile as tile
from concourse.bass import ts, ds

BF16 = mybir.dt.bfloat16
F32 = mybir.dt.float32
F32R = mybir.dt.float32r
FP8 = mybir.dt.float8e4
AF = mybir.ActivationFunctionType
ALU = mybir.AluOpType
PM = mybir.MatmulPerfMode
BF16NP = ml_dtypes.bfloat16

T, C, H, HD = 1024, 768, 12, 64
NCORES = 8
NEG = -60000.0

_CACHE = {}


def build_program():
    """Build the single-core SPMD Bass program."""
    nc = bacc.Bacc("TRN2", target_bir_lowering=False, debug=False)

    xT_d = nc.dram_tensor("xT", [C, T], BF16, kind="ExternalInput")
    wqk_d = nc.dram_tensor("wqk", [C, 2 * C], BF16, kind="ExternalInput")
    wv_d = nc.dram_tensor("wv", [C, C], BF16, kind="ExternalInput")
    wp_d = nc.dram_tensor("wp", [C, C], BF16, kind="ExternalInput")
    bqk_d = nc.dram_tensor("bqk", [128, 12], F32, kind="ExternalInput")
    bv_d = nc.dram_tensor("bv", [128, C], F32, kind="ExternalInput")
    bp_d = nc.dram_tensor("bp", [128, C], F32, kind="ExternalInput")
    qm_d = nc.dram_tensor("qm", [128, 8], F32, kind="ExternalInput")
    dmL_d = nc.dram_tensor("dmL", [128, 8, 128], BF16, kind="ExternalInput")
    id_d = nc.dram_tensor("id128", [128, 128], BF16, kind="ExternalInput")
    y_d = nc.dram_tensor("y", [T, C], F32, kind="ExternalOutput")

    with tile.TileContext(nc) as tc:
        with (
            tc.tile_pool(name="const", bufs=1) as cp,
            tc.tile_pool(name="ptp", bufs=4) as ptp,
            tc.tile_pool(name="sums", bufs=2) as sumsp,
            tc.tile_pool(name="bcsp", bufs=2) as bcsp,
            tc.tile_pool(name="otxp", bufs=2) as otxp,
            tc.tile_pool(name="ysb", bufs=2) as ysbp,
            tc.tile_pool(name="ps_mm", bufs=2, space="PSUM") as ps_mm,
            tc.tile_pool(name="ps_s", bufs=2, space="PSUM") as ps_s,
            tc.tile_pool(name="ps_o", bufs=2, space="PSUM") as ps_o,
        ):
            # ---------------- persistent SBUF tensors ----------------
            xT_sb = cp.tile([128, 6, T], BF16, name="xT_sb")
            wqk_sb = cp.tile([128, 6, 2 * C], BF16, name="wqk_sb")
            wv_sb = cp.tile([128, 6, C], BF16, name="wv_sb")
            wp_sb = cp.tile([128, 6, C], BF16, name="wp_sb")
            bqk_sb = cp.tile([128, 12], F32, name="bqk_sb")
            bv_sb = cp.tile([128, C], F32, name="bv_sb")
            bp_sb = cp.tile([128, C], F32, name="bp_sb")
            qm_sb = cp.tile([128, 8], F32, name="qm_sb")
            dmL_sb = cp.tile([128, 8, 128], BF16, name="dmL_sb")
            id_sb = cp.tile([128, 128], BF16, name="id_sb")
            ones_sb = cp.tile([128, 64], F32, name="ones_sb")
            onesr_sb = cp.tile([128, 64], F32R, name="onesr_sb")
            qk_sb = [cp.tile([128, T], BF16, name=f"qk{m}") for m in range(12)]
            v_sb = [cp.tile([128, 12 * 65], BF16, name=f"v{t}") for t in range(8)]
            ot_sb = cp.tile([128, 6, T], BF16, name="ot_sb")

            # ---------------- loads (priority order) ----------------
            xT_ap = xT_d[:, :].rearrange("(k p) t -> p k t", p=128)
            wqk_ap = wqk_d[:, :].rearrange("(k p) m -> p k m", p=128)
            wv_ap = wv_d[:, :].rearrange("(k p) m -> p k m", p=128)
            wp_ap = wp_d[:, :].rearrange("(k p) m -> p k m", p=128)

            # the gpsimd SWDGE queue issues its first DMA only after an ~8us
            # engine preamble (library load + semaphore barriers), while the
            # SP HWDGE queue starts within ~1us but gets starved once SWDGE
            # traffic ramps. So: the first-matmul chain rides SP, and
            # everything else rides SWDGE, which comes up during the
            # qk(0)/qk(6) window.
            nc.sync.dma_start(wqk_sb[:, :, 0:128], wqk_ap[:, :, 0:128])
            nc.sync.dma_start(bqk_sb[:], bqk_d[:, :])
            nc.sync.dma_start(xT_sb[:, :, 0:512], xT_ap[:, :, 0:512])
            nc.sync.dma_start(wqk_sb[:, :, 768:896], wqk_ap[:, :, 768:896])
            nc.sync.dma_start(xT_sb[:, :, 512:1024], xT_ap[:, :, 512:1024])
            nc.gpsimd.dma_start(wv_sb[:], wv_ap[:, :, :])
            nc.gpsimd.dma_start(bv_sb[:], bv_d[:, :])
            nc.gpsimd.dma_start(dmL_sb[:], dmL_d[:, :, :])
            nc.gpsimd.dma_start(id_sb[:], id_d[:, :])
            nc.gpsimd.dma_start(qm_sb[:], qm_d[:, :])
            for m in (1, 7, 2, 8, 3, 9, 4, 10, 5, 11):
                nc.gpsimd.dma_start(
                    wqk_sb[:, :, ts(m, 128)], wqk_ap[:, :, ts(m, 128)]
                )
            nc.gpsimd.dma_start(wp_sb[:], wp_ap[:, :, :])
            nc.gpsimd.dma_start(bp_sb[:], bp_d[:, :])
            # ones columns interleaved into V (produce softmax sums during PV)
            for t in range(8):
                nc.vector.memset(
                    v_sb[t].rearrange("p (h d) -> p h d", d=65)[:, :, 64:65], 1.0
                )
            nc.vector.memset(ones_sb[:], 1.0)
            nc.vector.tensor_copy(onesr_sb[:], ones_sb[:])

            # deferred softmax-normalize closures, flushed when the tensor
            # queue is deep (next qk window / proj filler)
            pending_norm = []

            def flush_norm():
                while pending_norm:
                    pending_norm.pop(0)()

            # ---------------- qk projections: one 128-col m-tile ----------------
            def emit_qk_j(m, j):
                ps = ps_mm.tile([128, 512], F32, name="psmm", tag="mm")
                for k in range(6):
                    nc.tensor.matmul(
                        ps[:],
                        wqk_sb[:, k, ts(m, 128)],
                        xT_sb[:, k, ts(j, 512)],
                        start=(k == 0),
                        stop=(k == 5),
                    )
                nc.vector.tensor_scalar_add(
                    qk_sb[m][:, ts(j, 512)], ps[:], bqk_sb[:, m : m + 1]
                )

            def emit_qk(m):
                # flush at entry: sums copies were issued right after the
                # strip's last PV, so the bc matmuls are ready to run and the
                # qk groups behind them keep the PE fed while DVE normalizes
                flush_norm()
                for j in range(2):
                    emit_qk_j(m, j)

            # ---------------- V = x @ W_v + bv for one key tile ----------------
            def emit_v(t):
                for c0, cw in ((0, 512), (512, 256)):
                    psv = ps_mm.tile([128, 512], F32, name="psv", tag="mm")
                    for k in range(6):
                        nc.tensor.matmul(
                            psv[:, :cw],
                            xT_sb[:, k, ts(t, 128)],
                            wv_sb[:, k, ds(c0, cw)],
                            start=(k == 0),
                            stop=(k == 5),
                        )
                    nh, h0 = cw // 64, c0 // 64
                    nc.vector.tensor_add(
                        v_sb[t].rearrange("p (h d) -> p h d", d=65)[
                            :, h0 : h0 + nh, 0:64
                        ],
                        psv[:, :cw].rearrange("p (h d) -> p h d", d=64),
                        bv_sb[:, ds(c0, cw)].rearrange("p (h d) -> p h d", d=64),
                    )

            # ---------------- proj: y tile = OT.T @ W_proj * qm + bp ----------------
            # k-interleaved across both column halves so the k=5 matmuls
            # (the only ones depending on the freshest ot rows) run last
            def proj(qt, tail=False):
                ysb = ysbp.tile([128, C], F32, name="ysb", tag="ysb")
                halves = ((0, 512), (512, 256))
                psy = {}
                for c0, cw in halves:
                    psy[c0] = ps_mm.tile([128, 512], F32, name="psy", tag="mm")
                for k in range(6):
                    for c0, cw in halves:
                        nc.tensor.matmul(
                            psy[c0][:, :cw],
                            ot_sb[:, k, ts(qt, 128)],
                            wp_sb[:, k, ds(c0, cw)],
                            start=(k == 0),
                            stop=(k == 5),
                        )
                for c0, cw in halves:
                    nc.vector.scalar_tensor_tensor(
                        out=ysb[:, ds(c0, cw)],
                        in0=psy[c0][:, :cw],
                        scalar=qm_sb[:, qt : qt + 1],
                        in1=bp_sb[:, ds(c0, cw)],
                        op0=ALU.mult,
                        op1=ALU.add,
                    )
                # tail y tiles go out via the scalar queue (idle there);
                # mid-kernel ones via sync to keep the exp stream clean
                eng = nc.scalar if tail else nc.sync
                eng.dma_start(y_d[ts(qt, 128), :], ysb[:])

            # ---------------- attention for one head pair, one query strip ----------------
            def attention(pr, sbi, filler=None, tail=False):
                q0 = sbi * 512
                nkt = 4 + 4 * sbi
                hs = (2 * pr, 2 * pr + 1)
                psO = {}
                for h in hs:
                    psO[h] = ps_o.tile([65, 512], F32, name="psO", tag="op")

                def emit_pv(kt_, ptt_, dc_, w_):
                    for i, h in enumerate(hs):
                        nc.tensor.matmul(
                            psO[h][:, ds(dc_, w_)],
                            v_sb[kt_][:, h * 65 : h * 65 + 65],
                            ptt_[:, ds(512 * i + dc_, w_)],
                            start=(kt_ == 0),
                            stop=(kt_ == nkt - 1),
                            skip_group_check=True,
                        )

                pend = []
                for kt in range(nkt):
                    dc = max(0, kt * 128 - q0)
                    w = 512 - dc
                    diag = kt * 128 >= q0
                    sp = ps_s.tile([128, 1024], F32, name="sp", tag="sp")
                    for i, h in enumerate(hs):
                        qp = (h % 2) * 64
                        nc.tensor.matmul(
                            sp[:, ds(512 * i + dc, w)],
                            qk_sb[6 + pr][qp : qp + 64, ts(kt, 128)],
                            qk_sb[pr][qp : qp + 64, ds(q0 + dc, w)],
                            start=True,
                            stop=not diag,
                            skip_group_check=diag,
                        )
                    if diag:
                        # fold the causal/key-valid mask for the diagonal
                        # 128x128 block additively into the S psum:
                        # sp[key, q] += dmL[q, kt, key] via dmL^T @ I
                        for i in range(2):
                            nc.tensor.matmul(
                                sp[:, ds(512 * i + dc, 128)],
                                dmL_sb[:, kt, :],
                                id_sb[:, :],
                                start=False,
                                stop=True,
                                skip_group_check=True,
                            )
                    ptt = ptp.tile([128, 1024], BF16, name="ptt", tag="ptt")
                    sp3 = sp[:].rearrange("p (i q) -> p i q", i=2)
                    pt3 = ptt[:].rearrange("p (i q) -> p i q", i=2)
                    nc.scalar.activation(
                        pt3[:, :, ds(dc, w)], sp3[:, :, ds(dc, w)], AF.Exp
                    )
                    if pend:
                        emit_pv(*pend.pop(0))
                    pend.append((kt, ptt, dc, w))
                    if filler is not None and kt in filler:
                        filler[kt]()
                while pend:
                    emit_pv(*pend.pop(0))

                # softmax-sum extraction right away (DVE is idle here and the
                # deferred bc matmul must not wait on it)
                sums = {}
                for h in hs:
                    st = sumsp.tile([65, 512], F32R, name="sums", tag="sums")
                    nc.vector.tensor_copy(st[64:65, :], psO[h][64:65, :])
                    sums[h] = st
                if tail:
                    return psO, sums

                # deferred normalize: OT = psO[0:64] / psO[64] per query col
                # (odd head first: its ot write needs an extra DMA hop)
                def norm():
                    for h in sorted(hs, reverse=True):
                        bc = ps_mm.tile([128, 512], F32, name="bc", tag="mm")
                        nc.tensor.matmul(
                            bc[0:64, :],
                            onesr_sb[64:65, 0:64],
                            sums[h][64:65, :],
                            start=True,
                            stop=True,
                        )
                        bcs = bcsp.tile([64, 512], F32, name="bcs", tag="bcs")
                        nc.vector.reciprocal_approx_fast(bcs[:], bc[0:64, :])
                        if h % 2 == 0:
                            nc.vector.tensor_mul(
                                ot_sb[0:64, pr, ds(q0, 512)], psO[h][0:64, :], bcs[:]
                            )
                        else:
                            otx = otxp.tile([64, 512], BF16, name="otx", tag="otx")
                            nc.vector.tensor_mul(otx[:], psO[h][0:64, :], bcs[:])
                            nc.sync.dma_start(
                                ot_sb[64:128, pr, ds(q0, 512)], otx[:]
                            )

                pending_norm.append(norm)

            # ---------------- schedule ----------------
            # j-granular head so each matmul group starts as soon as its
            # xT half lands; the four qk groups bridge the gpsimd preamble
            # until wv arrives for V
            emit_qk_j(0, 0)
            emit_qk_j(6, 0)
            emit_qk_j(0, 1)
            emit_qk_j(6, 1)
            for t in range(8):
                emit_v(t)
            for pr in range(6):
                if pr < 5:
                    emit_qk(pr + 1)  # flushes norm(pr-1, sbi=1)
                    attention(pr, 0)
                    emit_qk(7 + pr)  # flushes norm(pr, sbi=0)
                    attention(pr, 1)
                else:
                    attention(pr, 0, filler={1: flush_norm})  # norm(4,1)
                    psO5, sums5 = attention(
                        pr,
                        1,
                        tail=True,
                        filler={
                            # deep buffer at kt=0: norm(5,0) muls must beat
                            # this strip's first PV (psO buffer rotation)
                            0: lambda: (flush_norm(), proj(0), proj(1)),
                            3: lambda: proj(2),
                            6: lambda: proj(3),
                        },
                    )
            # ---------------- tail: chunked normalize + proj interleave ----------------
            h0t, h1t = 10, 11
            bcs5 = {}
            for h in (h1t, h0t):
                bc = ps_mm.tile([128, 512], F32, name="bc", tag="mm")
                nc.tensor.matmul(
                    bc[0:64, :],
                    onesr_sb[64:65, 0:64],
                    sums5[h][64:65, :],
                    start=True,
                    stop=True,
                )
                bcs5[h] = bcsp.tile([64, 512], F32, name="bcs", tag="bcs")
                nc.vector.reciprocal_approx_fast(bcs5[h][:], bc[0:64, :])
            otx5 = otxp.tile([64, 512], BF16, name="otx", tag="otx")
            for qt in range(4, 8):
                c0 = (qt - 4) * 128
                nc.vector.tensor_mul(
                    otx5[:, ds(c0, 128)],
                    psO5[h1t][0:64, ds(c0, 128)],
                    bcs5[h1t][:, ds(c0, 128)],
                )
                nc.sync.dma_start(
                    ot_sb[64:128, 5, ds(512 + c0, 128)], otx5[:, ds(c0, 128)]
                )
                nc.vector.tensor_mul(
                    ot_sb[0:64, 5, ds(512 + c0, 128)],
                    psO5[h0t][0:64, ds(c0, 128)],
                    bcs5[h0t][:, ds(c0, 128)],
                )
                proj(qt, tail=True)

    nc.compile()
    return nc


def _get_nc():
    if "nc" not in _CACHE:
        _CACHE["nc"] = build_program()
    return _CACHE["nc"]


def prep_core_inputs(x, mask, query_mask, W_attn, b_attn, W_proj, b_proj):
    """Host-side prep. Returns a list of per-core input dicts (one per batch
    element)."""
    scale = 1.0 / np.sqrt(HD)
    W_s = np.asarray(W_attn, np.float32).copy()
    W_s[:, :C] *= scale
    b_s = np.asarray(b_attn, np.float32).copy()
    b_s[:C] *= scale

    shared = {
        "wqk": W_s[:, : 2 * C].astype(BF16NP),
        "wv": W_s[:, 2 * C :].astype(BF16NP),
        "wp": np.asarray(W_proj, np.float32).astype(BF16NP),
        "bqk": np.ascontiguousarray(b_s[: 2 * C].reshape(12, 128).T),
        "bv": np.ascontiguousarray(
            np.broadcast_to(b_s[2 * C :], (128, C))
        ).astype(np.float32),
        "bp": np.ascontiguousarray(
            np.broadcast_to(np.asarray(b_proj, np.float32), (128, C))
        ),
        "id128": np.eye(128, dtype=BF16NP),
    }

    per_core = []
    for b in range(NCORES):
        xT = np.ascontiguousarray(np.asarray(x[b], np.float32).T).astype(BF16NP)
        qm = np.ascontiguousarray(
            np.asarray(query_mask[b, 0, :, 0], np.float32).reshape(8, 128).T
        )
        mb = np.asarray(mask[b, 0])  # [T, T] bool
        # additive diagonal-block masks: [q_within, qi, k_within]
        blocks = [
            np.where(
                mb[qi * 128 : (qi + 1) * 128, qi * 128 : (qi + 1) * 128],
                0.0,
                NEG,
            ).astype(np.float32)
            for qi in range(8)
        ]
        dmL = np.stack(blocks, axis=1).astype(BF16NP)  # [128, 8, 128]
        per_core.append({"xT": xT, "qm": qm, "dmL": dmL, **shared})
    return per_core


def run_on_cores(inputs, trace=False, **kw):
    from concourse.bass_utils import run_bass_kernel_spmd

    nc = _get_nc()
    in_maps = prep_core_inputs(**inputs)
    res = run_bass_kernel_spmd(
        nc, in_maps, core_ids=list(range(NCORES)), trace=trace, **kw
    )
    out = np.stack([res.results[b]["y"] for b in range(NCORES)], axis=0)
    return out.astype(np.float32), res


def kernel(**inputs) -> np.ndarray:
    out, _ = run_on_cores(inputs, trace=False)
    return out


# revision 36
# speedup vs baseline: 1.0694x; 1.0694x over previous
"""Bass/Trainium2 kernel for CausalSelfAttention (B=8, T=1024, C=768, H=12).

Sharding: data-parallel over batch. 8 cores, one batch element per core.
No collectives. Each core runs an identical SPMD program on its own slice.

Per-core layouts (host-prepared):
  xT   [768, 1024] bf16   x[b].T
  wqk  [768, 1536] bf16   W_attn[:, :1536], Q columns pre-scaled by 1/sqrt(64)
  wv   [768, 768]  bf16   W_attn[:, 1536:]
  wp   [768, 768]  bf16   W_proj
  bqk  [128, 12]  f32     b_attn[:1536] per-tile columns (Q part pre-scaled)
  bv   [128, 768] f32     b_attn[1536:] broadcast over partitions
  bp   [128, 768] f32     b_proj broadcast over partitions
  qm   [128, 8]   f32     query_mask as per-partition columns per q-tile
  dmL  [128, 8, 128] bf16 diagonal-block ADDITIVE masks (0 / -60000), with
                          partition = query-within-block, free = key-within
  id128 [128, 128] bf16   identity (rhs of the mask-add matmuls)
Output: y [1024, 768] f32 per core.

Schedule (keep the tensor engine continuously busy so its DVFS p-state
ramps to full clock and never drops):
  - fine-grained priority DMA: first-matmul deps (wqk m-block 0, xT j0)
    land first; the PE starts ~2us in
  - qk projections for (m=0,6), V for all 8 key tiles, then pairs
    pr=0..5: [emit_qk(pr+1) | flush norm(pr-1,1)], attention(pr,0),
    [emit_qk(7+pr) | flush norm(pr,0)], attention(pr,1)
  - attention software-pipelines PV one key-tile behind the exp so the
    PE never waits on the scalar engine
  - causal masking of diagonal blocks is ADDITIVE, folded into the S
    psum by one extra 128-row matmul (dmL^T @ I) per head -- no
    cross-engine op sits between exp and PV
  - both heads of a pair share one 2-bank [128,1024] S psum; one strided
    exp covers both heads (halves scalar-engine instruction count)
  - softmax normalization (copy sums -> PE broadcast -> reciprocal ->
    multiply) is DEFERRED and flushed at the next qk-projection window,
    when the tensor queue holds >1us of work, so the broadcast matmul
    and the psO buffer rotation never stall the PE
  - qkv bias-adds on DVE to keep the scalar engine exp-only
  - output projection qt 0-3 interleaved into the last pair's sbi=1
    attention; only qt 4-7 in the tail
"""

import sys

if "/opt/trn_rl_repo" not in sys.path:
    sys.path.insert(0, "/opt/trn_rl_repo")

import numpy as np
import ml_dtypes

import concourse.bass as bass
import concourse.bacc as bacc
import concourse.mybir as mybir
import concourse.tile as tile
from concourse.bass import ts, ds

BF16 = mybir.dt.bfloat16
F32 = mybir.dt.float32
F32R = mybir.dt.float32r
FP8 = mybir.dt.float8e4
AF = mybir.ActivationFunctionType
ALU = mybir.AluOpType
PM = mybir.MatmulPerfMode
BF16NP = ml_dtypes.bfloat16

T, C, H, HD = 1024, 768, 12, 64
NCORES = 8
NEG = -60000.0

_CACHE = {}


def build_program():
    """Build the single-core SPMD Bass program."""
    nc = bacc.Bacc("TRN2", target_bir_lowering=False, debug=False)

    xT_d = nc.dram_tensor("xT", [C, T], BF16, kind="ExternalInput")
    wqk_d = nc.dram_tensor("wqk", [C, 2 * C], BF16, kind="ExternalInput")
    wv_d = nc.dram_tensor("wv", [C, C], BF16, kind="ExternalInput")
    wp_d = nc.dram_tensor("wp", [C, C], BF16, kind="ExternalInput")
    bqk_d = nc.dram_tensor("bqk", [128, 12], F32, kind="ExternalInput")
    bv_d = nc.dram_tensor("bv", [128, C], F32, kind="ExternalInput")
    bp_d = nc.dram_tensor("bp", [128, C], F32, kind="ExternalInput")
    qm_d = nc.dram_tensor("qm", [128, 8], F32, kind="ExternalInput")
    dmL_d = nc.dram_tensor("dmL", [128, 8, 128], BF16, kind="ExternalInput")
    id_d = nc.dram_tensor("id128", [128, 128], BF16, kind="ExternalInput")
    y_d = nc.dram_tensor("y", [T, C], F32, kind="ExternalOutput")

    with tile.TileContext(nc) as tc:
        with (
            tc.tile_pool(name="const", bufs=1) as cp,
            tc.tile_pool(name="ptp", bufs=4) as ptp,
            tc.tile_pool(name="sums", bufs=2) as sumsp,
            tc.tile_pool(name="bcsp", bufs=2) as bcsp,
            tc.tile_pool(name="otxp", bufs=2) as otxp,
            tc.tile_pool(name="ysb", bufs=2) as ysbp,
            tc.tile_pool(name="ps_mm", bufs=2, space="PSUM") as ps_mm,
            tc.tile_pool(name="ps_s", bufs=2, space="PSUM") as ps_s,
            tc.tile_pool(name="ps_o", bufs=2, space="PSUM") as ps_o,
        ):
            # ---------------- persistent SBUF tensors ----------------
            xT_sb = cp.tile([128, 6, T], BF16, name="xT_sb")
            wqk_sb = cp.tile([128, 6, 2 * C], BF16, name="wqk_sb")
            wv_sb = cp.tile([128, 6, C], BF16, name="wv_sb")
            wp_sb = cp.tile([128, 6, C], BF16, name="wp_sb")
            bqk_sb = cp.tile([128, 12], F32, name="bqk_sb")
            bv_sb = cp.tile([128, C], F32, name="bv_sb")
            bp_sb = cp.tile([128, C], F32, name="bp_sb")
            qm_sb = cp.tile([128, 8], F32, name="qm_sb")
            dmL_sb = cp.tile([128, 8, 128], BF16, name="dmL_sb")
            id_sb = cp.tile([128, 128], BF16, name="id_sb")
            ones_sb = cp.tile([128, 64], F32, name="ones_sb")
            onesr_sb = cp.tile([128, 64], F32R, name="onesr_sb")
            qk_sb = [cp.tile([128, T], BF16, name=f"qk{m}") for m in range(12)]
            v_sb = [cp.tile([128, 12 * 65], BF16, name=f"v{t}") for t in range(8)]
            ot_sb = cp.tile([128, 6, T], BF16, name="ot_sb")

            # ---------------- loads (priority order) ----------------
            xT_ap = xT_d[:, :].rearrange("(k p) t -> p k t", p=128)
            wqk_ap = wqk_d[:, :].rearrange("(k p) m -> p k m", p=128)
            wv_ap = wv_d[:, :].rearrange("(k p) m -> p k m", p=128)
            wp_ap = wp_d[:, :].rearrange("(k p) m -> p k m", p=128)

            # the gpsimd SWDGE queue issues its first DMA only after an ~8us
            # engine preamble (library load + semaphore barriers), while the
            # SP HWDGE queue starts within ~1us but gets starved once SWDGE
            # traffic ramps. So: the first-matmul chain rides SP, and
            # everything else rides SWDGE, which comes up during the
            # qk(0)/qk(6) window.
            nc.sync.dma_start(wqk_sb[:, :, 0:128], wqk_ap[:, :, 0:128])
            nc.sync.dma_start(bqk_sb[:], bqk_d[:, :])
            nc.sync.dma_start(wqk_sb[:, :, 768:896], wqk_ap[:, :, 768:896])
            nc.scalar.dma_start(xT_sb[:, :, 0:512], xT_ap[:, :, 0:512])
            nc.scalar.dma_start(xT_sb[:, :, 512:1024], xT_ap[:, :, 512:1024])
            nc.gpsimd.dma_start(wv_sb[:], wv_ap[:, :, :])
            nc.gpsimd.dma_start(bv_sb[:], bv_d[:, :])
            nc.gpsimd.dma_start(dmL_sb[:], dmL_d[:, :, :])
            nc.gpsimd.dma_start(id_sb[:], id_d[:, :])
            nc.gpsimd.dma_start(qm_sb[:], qm_d[:, :])
            for m in (1, 7, 2, 8, 3, 9, 4, 10, 5, 11):
                nc.gpsimd.dma_start(
                    wqk_sb[:, :, ts(m, 128)], wqk_ap[:, :, ts(m, 128)]
                )
            nc.gpsimd.dma_start(wp_sb[:], wp_ap[:, :, :])
            nc.gpsimd.dma_start(bp_sb[:], bp_d[:, :])
            # ones columns interleaved into V (produce softmax sums during PV)
            for t in range(8):
                nc.vector.memset(
                    v_sb[t].rearrange("p (h d) -> p h d", d=65)[:, :, 64:65], 1.0
                )
            nc.vector.memset(ones_sb[:], 1.0)
            nc.vector.tensor_copy(onesr_sb[:], ones_sb[:])

            # deferred softmax-normalize closures, flushed when the tensor
            # queue is deep (next qk window / proj filler)
            pending_norm = []

            def flush_norm():
                while pending_norm:
                    pending_norm.pop(0)()

            # ---------------- qk projections: one 128-col m-tile ----------------
            def emit_qk_j(m, j):
                ps = ps_mm.tile([128, 512], F32, name="psmm", tag="mm")
                for k in range(6):
                    nc.tensor.matmul(
                        ps[:],
                        wqk_sb[:, k, ts(m, 128)],
                        xT_sb[:, k, ts(j, 512)],
                        start=(k == 0),
                        stop=(k == 5),
                    )
                nc.vector.tensor_scalar_add(
                    qk_sb[m][:, ts(j, 512)], ps[:], bqk_sb[:, m : m + 1]
                )

            def emit_qk(m):
                # flush at entry: sums copies were issued right after the
                # strip's last PV, so the bc matmuls are ready to run and the
                # qk groups behind them keep the PE fed while DVE normalizes
                flush_norm()
                for j in range(2):
                    emit_qk_j(m, j)

            # ---------------- V = x @ W_v + bv for one key tile ----------------
            def emit_v(t):
                for c0, cw in ((0, 512), (512, 256)):
                    psv = ps_mm.tile([128, 512], F32, name="psv", tag="mm")
                    for k in range(6):
                        nc.tensor.matmul(
                            psv[:, :cw],
                            xT_sb[:, k, ts(t, 128)],
                            wv_sb[:, k, ds(c0, cw)],
                            start=(k == 0),
                            stop=(k == 5),
                        )
                    nh, h0 = cw // 64, c0 // 64
                    nc.vector.tensor_add(
                        v_sb[t].rearrange("p (h d) -> p h d", d=65)[
                            :, h0 : h0 + nh, 0:64
                        ],
                        psv[:, :cw].rearrange("p (h d) -> p h d", d=64),
                        bv_sb[:, ds(c0, cw)].rearrange("p (h d) -> p h d", d=64),
                    )

            # ---------------- proj: y tile = OT.T @ W_proj * qm + bp ----------------
            # k-interleaved across both column halves so the k=5 matmuls
            # (the only ones depending on the freshest ot rows) run last
            def proj(qt, tail=False):
                ysb = ysbp.tile([128, C], F32, name="ysb", tag="ysb")
                halves = ((0, 512), (512, 256))
                psy = {}
                for c0, cw in halves:
                    psy[c0] = ps_mm.tile([128, 512], F32, name="psy", tag="mm")
                for k in range(6):
                    for c0, cw in halves:
                        nc.tensor.matmul(
                            psy[c0][:, :cw],
                            ot_sb[:, k, ts(qt, 128)],
                            wp_sb[:, k, ds(c0, cw)],
                            start=(k == 0),
                            stop=(k == 5),
                        )
                for c0, cw in halves:
                    nc.vector.scalar_tensor_tensor(
                        out=ysb[:, ds(c0, cw)],
                        in0=psy[c0][:, :cw],
                        scalar=qm_sb[:, qt : qt + 1],
                        in1=bp_sb[:, ds(c0, cw)],
                        op0=ALU.mult,
                        op1=ALU.add,
                    )
                # tail y tiles go out via the scalar queue (idle there);
                # mid-kernel ones via sync to keep the exp stream clean
                eng = nc.scalar if tail else nc.sync
                eng.dma_start(y_d[ts(qt, 128), :], ysb[:])

            # ---------------- attention for one head pair, one query strip ----------------
            def attention(pr, sbi, filler=None, tail=False):
                q0 = sbi * 512
                nkt = 4 + 4 * sbi
                hs = (2 * pr, 2 * pr + 1)
                psO = {}
                for h in hs:
                    psO[h] = ps_o.tile([65, 512], F32, name="psO", tag="op")

                def emit_pv(kt_, ptt_, dc_, w_):
                    for i, h in enumerate(hs):
                        nc.tensor.matmul(
                            psO[h][:, ds(dc_, w_)],
                            v_sb[kt_][:, h * 65 : h * 65 + 65],
                            ptt_[:, ds(512 * i + dc_, w_)],
                            start=(kt_ == 0),
                            stop=(kt_ == nkt - 1),
                            skip_group_check=True,
                        )

                pend = []
                for kt in range(nkt):
                    dc = max(0, kt * 128 - q0)
                    w = 512 - dc
                    diag = kt * 128 >= q0
                    sp = ps_s.tile([128, 1024], F32, name="sp", tag="sp")
                    for i, h in enumerate(hs):
                        qp = (h % 2) * 64
                        nc.tensor.matmul(
                            sp[:, ds(512 * i + dc, w)],
                            qk_sb[6 + pr][qp : qp + 64, ts(kt, 128)],
                            qk_sb[pr][qp : qp + 64, ds(q0 + dc, w)],
                            start=True,
                            stop=not diag,
                            skip_group_check=diag,
                        )
                    if diag:
                        # fold the causal/key-valid mask for the diagonal
                        # 128x128 block additively into the S psum:
                        # sp[key, q] += dmL[q, kt, key] via dmL^T @ I
                        for i in range(2):
                            nc.tensor.matmul(
                                sp[:, ds(512 * i + dc, 128)],
                                dmL_sb[:, kt, :],
                                id_sb[:, :],
                                start=False,
                                stop=True,
                                skip_group_check=True,
                            )
                    ptt = ptp.tile([128, 1024], BF16, name="ptt", tag="ptt")
                    sp3 = sp[:].rearrange("p (i q) -> p i q", i=2)
                    pt3 = ptt[:].rearrange("p (i q) -> p i q", i=2)
                    nc.scalar.activation(
                        pt3[:, :, ds(dc, w)], sp3[:, :, ds(dc, w)], AF.Exp
                    )
                    if pend:
                        emit_pv(*pend.pop(0))
                    pend.append((kt, ptt, dc, w))
                    if filler is not None and kt in filler:
                        filler[kt]()
                while pend:
                    emit_pv(*pend.pop(0))

                # softmax-sum extraction right away (DVE is idle here and the
                # deferred bc matmul must not wait on it)
                sums = {}
                for h in hs:
                    st = sumsp.tile([65, 512], F32R, name="sums", tag="sums")
                    nc.vector.tensor_copy(st[64:65, :], psO[h][64:65, :])
                    sums[h] = st
                if tail:
                    return psO, sums

                # deferred normalize: OT = psO[0:64] / psO[64] per query col
                # (odd head first: its ot write needs an extra DMA hop)
                def norm():
                    for h in sorted(hs, reverse=True):
                        bc = ps_mm.tile([128, 512], F32, name="bc", tag="mm")
                        nc.tensor.matmul(
                            bc[0:64, :],
                            onesr_sb[64:65, 0:64],
                            sums[h][64:65, :],
                            start=True,
                            stop=True,
                        )
                        bcs = bcsp.tile([64, 512], F32, name="bcs", tag="bcs")
                        nc.vector.reciprocal_approx_fast(bcs[:], bc[0:64, :])
                        if h % 2 == 0:
                            nc.vector.tensor_mul(
                                ot_sb[0:64, pr, ds(q0, 512)], psO[h][0:64, :], bcs[:]
                            )
                        else:
                            otx = otxp.tile([64, 512], BF16, name="otx", tag="otx")
                            nc.vector.tensor_mul(otx[:], psO[h][0:64, :], bcs[:])
                            nc.sync.dma_start(
                                ot_sb[64:128, pr, ds(q0, 512)], otx[:]
                            )

                pending_norm.append(norm)

            # ---------------- schedule ----------------
            # j-granular head so each matmul group starts as soon as its
            # xT half lands; V t0-3 only touch the first xT half and fill
            # the wait for xT j1
            emit_qk_j(0, 0)
            emit_qk_j(6, 0)
            for t in range(4):
                emit_v(t)
            emit_qk_j(0, 1)
            emit_qk_j(6, 1)
            for t in range(4, 8):
                emit_v(t)
            for pr in range(6):
                if pr < 5:
                    emit_qk(pr + 1)  # flushes norm(pr-1, sbi=1)
                    attention(pr, 0)
                    emit_qk(7 + pr)  # flushes norm(pr, sbi=0)
                    attention(pr, 1)
                else:
                    attention(pr, 0, filler={1: flush_norm})  # norm(4,1)
                    psO5, sums5 = attention(
                        pr,
                        1,
                        tail=True,
                        filler={
                            # deep buffer at kt=0: norm(5,0) muls must beat
                            # this strip's first PV (psO buffer rotation)
                            0: lambda: (flush_norm(), proj(0), proj(1)),
                            3: lambda: proj(2),
                            6: lambda: proj(3),
                        },
                    )
            # ---------------- tail: chunked normalize + proj interleave ----------------
            h0t, h1t = 10, 11
            bcs5 = {}
            for h in (h1t, h0t):
                bc = ps_mm.tile([128, 512], F32, name="bc", tag="mm")
                nc.tensor.matmul(
                    bc[0:64, :],
                    onesr_sb[64:65, 0:64],
                    sums5[h][64:65, :],
                    start=True,
                    stop=True,
                )
                bcs5[h] = bcsp.tile([64, 512], F32, name="bcs", tag="bcs")
                nc.vector.reciprocal_approx_fast(bcs5[h][:], bc[0:64, :])
            otx5 = otxp.tile([64, 512], BF16, name="otx", tag="otx")
            for qt in range(4, 8):
                c0 = (qt - 4) * 128
                nc.vector.tensor_mul(
                    otx5[:, ds(c0, 128)],
                    psO5[h1t][0:64, ds(c0, 128)],
                    bcs5[h1t][:, ds(c0, 128)],
                )
                nc.sync.dma_start(
                    ot_sb[64:128, 5, ds(512 + c0, 128)], otx5[:, ds(c0, 128)]
                )
                nc.vector.tensor_mul(
                    ot_sb[0:64, 5, ds(512 + c0, 128)],
                    psO5[h0t][0:64, ds(c0, 128)],
                    bcs5[h0t][:, ds(c0, 128)],
                )
                proj(qt, tail=True)

    nc.compile()
    return nc


def _get_nc():
    if "nc" not in _CACHE:
        _CACHE["nc"] = build_program()
    return _CACHE["nc"]


def prep_core_inputs(x, mask, query_mask, W_attn, b_attn, W_proj, b_proj):
    """Host-side prep. Returns a list of per-core input dicts (one per batch
    element)."""
    scale = 1.0 / np.sqrt(HD)
    W_s = np.asarray(W_attn, np.float32).copy()
    W_s[:, :C] *= scale
    b_s = np.asarray(b_attn, np.float32).copy()
    b_s[:C] *= scale

    shared = {
        "wqk": W_s[:, : 2 * C].astype(BF16NP),
        "wv": W_s[:, 2 * C :].astype(BF16NP),
        "wp": np.asarray(W_proj, np.float32).astype(BF16NP),
        "bqk": np.ascontiguousarray(b_s[: 2 * C].reshape(12, 128).T),
        "bv": np.ascontiguousarray(
            np.broadcast_to(b_s[2 * C :], (128, C))
        ).astype(np.float32),
        "bp": np.ascontiguousarray(
            np.broadcast_to(np.asarray(b_proj, np.float32), (128, C))
        ),
        "id128": np.eye(128, dtype=BF16NP),
    }

    per_core = []
    for b in range(NCORES):
        xT = np.ascontiguousarray(np.asarray(x[b], np.float32).T).astype(BF16NP)
        qm = np.ascontiguousarray(
            np.asarray(query_mask[b, 0, :, 0], np.float32).reshape(8, 128).T
        )
        mb = np.asarray(mask[b, 0])  # [T, T] bool
        # additive diagonal-block masks: [q_within, qi, k_within]
        blocks = [
            np.where(
                mb[qi * 128 : (qi + 1) * 128, qi * 128 : (qi + 1) * 128],
                0.0,
                NEG,
            ).astype(np.float32)
            for qi in range(8)
        ]
        dmL = np.stack(blocks, axis=1).astype(BF16NP)  # [128, 8, 128]
        per_core.append({"xT": xT, "qm": qm, "dmL": dmL, **shared})
    return per_core


def run_on_cores(inputs, trace=False, **kw):
    from concourse.bass_utils import run_bass_kernel_spmd

    nc = _get_nc()
    in_maps = prep_core_inputs(**inputs)
    res = run_bass_kernel_spmd(
        nc, in_maps, core_ids=list(range(NCORES)), trace=trace, **kw
    )
    out = np.stack([res.results[b]["y"] for b in range(NCORES)], axis=0)
    return out.astype(np.float32), res


def kernel(**inputs) -> np.ndarray:
    out, _ = run_on_cores(inputs, trace=False)
    return out
